# revision 5
# baseline (speedup 1.0000x reference)
"""AnoNAViLa forward kernel for 8 TRN2 NeuronCores (data-parallel over batch).

Math (per branch):
  sims = (img @ text.T) * scale;  w = softmax(sims);  e = exp(w)
  x = concat([img_rep, text * e[..., None]], -1)
  h = relu(x @ W1 + b1); h = relu(h @ W2 + b2); h = h @ W3 + b3
  out = h.mean(axis=1)

Key algebraic restructuring (exact, up to fp assoc):
  x @ W1 = img @ W1[:D] + e[b,n] * (text @ W1[D:])      (rank-1 per (b,n))
  mean_n (h2 @ W3 + b3) = (mean_n h2) @ W3 + b3          (mean before layer 3)
"""
import sys

sys.path.insert(0, "/opt/trn_rl_repo")

from contextlib import ExitStack

import numpy as np
import orjson

import concourse.bass as bass
import concourse.mybir as mybir
import concourse.tile as tile
from concourse.bass import ds, ts
from concourse.bass_utils import run_bass_kernel_spmd

F32 = mybir.dt.float32
BF16 = mybir.dt.bfloat16
AF = mybir.ActivationFunctionType
ALU = mybir.AluOpType

NC = 8
B, N, D = 1024, 96, 512
BL = B // NC  # 128 rows per core
NG = N // 4  # 24 quad groups


# ---------------------------------------------------------------------------
# This walrus build rejects instructions with >1 semaphore wait/update
# ("Too many sync wait commands").  Split extras onto chained NoOps on the
# same engine (streams are in-order, so sequential waits == combined wait).
_bir_patch_installed = False


def _split_multi_sync(bir_json: bytes) -> bytes:
    d = orjson.loads(bir_json)
    ctr = [0]

    def mk_nop(inst, wait=None, update=None):
        ctr[0] += 1
        return {
            "debug": inst.get("debug", 0),
            "engine": inst["engine"],
            "ins": [],
            "outs": [],
            "name": f"{inst['name']}__ssplit{ctr[0]}",
            "opcode": "NoOp",
            "sync_info": {
                "on_update": [update] if update else [],
                "on_wait": [wait] if wait else [],
            },
        }

    changed = False
    for fn in d["functions"]:
        for bb in fn["blocks"]:
            new_insts = []
            for inst in bb["instructions"]:
                si = inst.get("sync_info")
                pre, post = [], []
                if si:
                    waits = si.get("on_wait") or []
                    if len(waits) > 1:
                        pre = [mk_nop(inst, wait=w) for w in waits[:-1]]
                        si["on_wait"] = [waits[-1]]
                        changed = True
                    upds = si.get("on_update") or []
                    if len(upds) > 1:
                        post = [mk_nop(inst, update=u) for u in upds[1:]]
                        si["on_update"] = [upds[0]]
                        changed = True
                new_insts.extend(pre)
                new_insts.append(inst)
                new_insts.extend(post)
            bb["instructions"] = new_insts
    return orjson.dumps(d) if changed else bir_json


def _install_bir_patch():
    global _bir_patch_installed
    if _bir_patch_installed:
        return
    _bir_patch_installed = True
    import concourse.bass_utils as bu
    import concourse.bass2jax as b2j

    orig = bu.compile_bir_kernel

    def patched(bir_json, tmpdir, neff_name="file.neff"):
        return orig(_split_multi_sync(bir_json), tmpdir, neff_name)

    bu.compile_bir_kernel = patched
    b2j.compile_bir_kernel = patched


# ---------------------------------------------------------------------------
def build_graph(scale: float, has_b1: bool, has_b2: bool, has_b3: bool) -> bass.Bass:
    nc = bass.Bass()

    img_ext = nc.declare_dram_parameter("img", [BL, D], F32, isOutput=False)
    tn_ext = nc.declare_dram_parameter("tn", [N, D], F32, isOutput=False)
    ta_ext = nc.declare_dram_parameter("ta", [N, D], F32, isOutput=False)
    W1_ext = nc.declare_dram_parameter("W1", [2 * D, D], F32, isOutput=False)
    b1_ext = nc.declare_dram_parameter("b1", [D], F32, isOutput=False)
    W2_ext = nc.declare_dram_parameter("W2", [D, D // 2], F32, isOutput=False)
    b2_ext = nc.declare_dram_parameter("b2", [D // 2], F32, isOutput=False)
    W3_ext = nc.declare_dram_parameter("W3", [D // 2, D // 4], F32, isOutput=False)
    b3_ext = nc.declare_dram_parameter("b3", [D // 4], F32, isOutput=False)
    idf_ext = nc.declare_dram_parameter("idf", [128, 128], F32, isOutput=False)
    idb_ext = nc.declare_dram_parameter("idb", [128, 128], BF16, isOutput=False)
    zer_ext = nc.declare_dram_parameter("zer", [NG * 2048], BF16, isOutput=False)
    out_ext = nc.declare_dram_parameter("out", [2, D // 4, BL], F32, isOutput=True)
    # DRAM bounce buffers for the cross-partition flatten of eT / tW
    eT_dram = nc.dram_tensor("eT_dram", [2, N * BL], BF16)
    tW_dram = nc.dram_tensor("tW_dram", [2, N * D], BF16)
    # block-diagonal selector: E4_dram[br, g] is [4, 512] row-major with
    # E4[k, k*128:(k+1)*128] = eT[4g+k, :] and zeros elsewhere
    E4_dram = nc.dram_tensor("E4_dram", [2, NG * 2048], BF16)

    with tile.TileContext(nc) as tc, ExitStack() as ctx:
        const = ctx.enter_context(tc.tile_pool(name="const", bufs=1))
        work = ctx.enter_context(tc.tile_pool(name="work", bufs=2))
        hpool = ctx.enter_context(tc.tile_pool(name="hpool", bufs=3))
        psH1 = ctx.enter_context(tc.tile_pool(name="psH1", bufs=2, space="PSUM"))
        psH2 = ctx.enter_context(tc.tile_pool(name="psH2", bufs=2, space="PSUM"))

        # ---- zero-padded K=128 selector tiles, one pair per branch ---------
        # rows 0..3 hold data; rows 4..127 zeroed once (as f32 view: GpSimd
        # memset cost is per-element, halving elements halves time)
        E4p = [const.tile([128, NG * 512], BF16, tag=f"E4p{i}", name=f"E4p{i}")
               for i in range(2)]
        tW4p = [const.tile([128, NG * 512], BF16, tag=f"tW4p{i}", name=f"tW4p{i}")
                for i in range(2)]
        for i in range(2):
            nc.gpsimd.memset(E4p[i][:].bitcast(F32), 0.0)
            nc.gpsimd.memset(tW4p[i][:].bitcast(F32), 0.0)

        # ---- DMA loads: critical tensors first, spread across queues -------
        I_s = const.tile([128, 128], F32)
        nc.sync.dma_start(out=I_s[:], in_=idf_ext[:, :])
        img_s = const.tile([BL, D], F32)
        nc.sync.dma_start(out=img_s[:], in_=img_ext[:, :])
        tn_s = const.tile([N, D], F32)
        nc.sync.dma_start(out=tn_s[:], in_=tn_ext[:, :])
        ta_s = const.tile([N, D], F32)
        nc.sync.dma_start(out=ta_s[:], in_=ta_ext[:, :])
        for br0 in range(2):
            nc.sync.dma_start(out=E4_dram[br0, :], in_=zer_ext[:])
        I_bf = const.tile([128, 128], BF16)
        nc.sync.dma_start(out=I_bf[:], in_=idb_ext[:, :])

        W1_s = const.tile([128, 8, D], F32)  # W1_s[p, c, :] = W1[c*128+p, :]
        nc.scalar.dma_start(
            out=W1_s[:], in_=W1_ext[:, :].rearrange("(c p) d -> p c d", p=128)
        )
        W2_s = const.tile([128, 4, D // 2], F32)
        nc.gpsimd.dma_start(
            out=W2_s[:], in_=W2_ext[:, :].rearrange("(c p) d -> p c d", p=128)
        )
        W3_s = const.tile([128, 2, D // 4], F32)
        nc.gpsimd.dma_start(
            out=W3_s[:], in_=W3_ext[:, :].rearrange("(c p) d -> p c d", p=128)
        )
        if has_b1:
            b1t = const.tile([128, 4], F32)
            nc.gpsimd.dma_start(out=b1t[:], in_=b1_ext[:].rearrange("(c p) -> p c", p=128))
        if has_b2:
            # b2 as a single-partition row (for a rank-1 bias matmul into PSUM)
            b2row = const.tile([1, D // 2], F32)
            nc.sync.dma_start(out=b2row[:], in_=b2_ext[None, :])
            b2row_bf = const.tile([1, D // 2], BF16)
            nc.gpsimd.tensor_copy(b2row_bf[:], b2row[:])
            ones_bf = const.tile([1, D], BF16)
            nc.gpsimd.memset(ones_bf[:], 1.0)
        if has_b3:
            b3t = const.tile([128, 1], F32)
            nc.gpsimd.dma_start(out=b3t[:], in_=b3_ext[:].rearrange("(c p) -> p c", p=128))

        # ---- imgT: [d-part, b-free], f32 (sims) + bf16 (aT4) copies --------
        imgT_f = const.tile([128, 4, BL], F32)
        imgT_bf = const.tile([128, 4, BL], BF16)
        for c in range(4):
            pt = psH1.tile([128, 128], F32, tag="h1p")
            nc.tensor.transpose(pt[:], img_s[:, ts(c, 128)], I_s[:])
            nc.scalar.copy(imgT_f[:, c, :], pt[:])
            nc.vector.tensor_copy(imgT_bf[:, c, :], pt[:])

        # ---- W1 halves as bf16 (scalar/vector split, overlaps DMA waits) ---
        W1t_bf = const.tile([128, 4, D], BF16)
        nc.scalar.copy(W1t_bf[:], W1_s[:, 0:4, :])
        W1b_bf = const.tile([128, 4, D], BF16)
        nc.vector.tensor_copy(W1b_bf[:], W1_s[:, 4:8, :])
        W2_bf = const.tile([128, 4, D // 2], BF16)
        nc.scalar.copy(W2_bf[:], W2_s[:])

        # ---- aT4: img@W1_top + b1, transposed, replicated 4x along free ----
        aT4_s = const.tile([128, 4, 512], BF16)
        for co in range(4):
            pa = psH2.tile([128, 128], F32, tag="h2p")
            for ci in range(4):
                nc.tensor.matmul(
                    pa[:], W1t_bf[:, ci, ts(co, 128)], imgT_bf[:, ci, :],
                    start=(ci == 0), stop=(ci == 3),
                )
            if has_b1:
                nc.scalar.activation(
                    aT4_s[:, co, 0:128], pa[:], AF.Identity, bias=b1t[:, co : co + 1]
                )
            else:
                nc.scalar.copy(aT4_s[:, co, 0:128], pa[:])
            # replicate 4x along free via SBUF->SBUF broadcast DMA
            rep_src = aT4_s[:, co, 0:128]
            nc.gpsimd.dma_start(
                out=aT4_s[:, co, ds(128, 384)],
                in_=bass.AP(tensor=rep_src.tensor, offset=rep_src.offset,
                            ap=[rep_src.ap[0], [0, 3], [1, 128]]),
            )

        def branch_prologue(br, text_s):
            # textT: [d-part, n-free] (f32 for sims, bf16 for the tW matmul)
            textT_s = work.tile([128, 4, N], F32, tag="textT")
            textT_bf = work.tile([128, 4, N], BF16, tag="textT_bf")
            for c in range(4):
                pt = psH1.tile([128, N], F32, tag="h1p")
                nc.tensor.transpose(pt[:], text_s[:, ts(c, 128)], I_s[:N, :N])
                nc.scalar.copy(textT_s[:, c, :], pt[:])
                nc.vector.tensor_copy(textT_bf[:, c, :], pt[:])

            # sims = img @ text.T (scale folded into the exp activation)
            ps_sims = psH2.tile([BL, N], F32, tag="h2p")
            for c in range(4):
                nc.tensor.matmul(
                    ps_sims[:], imgT_f[:, c, :], textT_s[:, c, :],
                    start=(c == 0), stop=(c == 3),
                )
            # softmax over n then e = exp(w), all rowwise
            negmax = work.tile([BL, 1], F32, tag="negmax")
            nc.vector.tensor_reduce(
                negmax[:], ps_sims[:], axis=mybir.AxisListType.X, op=ALU.max,
                negate=True,
            )
            nb = work.tile([BL, 1], F32, tag="nb")
            nc.vector.tensor_scalar_mul(nb[:], negmax[:], float(scale))
            E_s = work.tile([BL, N], F32, tag="E_s")
            ssum = work.tile([BL, 1], F32, tag="ssum")
            nc.scalar.activation(
                E_s[:], ps_sims[:], AF.Exp, bias=nb[:, 0:1], scale=float(scale),
                accum_out=ssum[:, 0:1],
            )
            rr = work.tile([BL, 1], F32, tag="rr")
            nc.vector.reciprocal(rr[:], ssum[:])
            e_x = work.tile([BL, N], F32, tag="e_x")
            nc.scalar.activation(e_x[:], E_s[:], AF.Exp, scale=rr[:, 0:1])

            # eT (bf16), flatten rows onto partition 0 via DRAM bounce
            pe_t = psH1.tile([N, BL], F32, tag="h1p")
            nc.tensor.transpose(pe_t[:], e_x[:], I_s[:])
            eT_bf = work.tile([N, BL], BF16, tag="eT_bf")
            nc.scalar.copy(eT_bf[:], pe_t[:])
            # scatter e-rows onto block diagonals, straight SBUF -> DRAM
            dst = E4_dram[br, 0:1]
            dst_ap = bass.AP(tensor=dst.tensor, offset=dst.offset,
                             ap=[[2048, NG], [640, 4], [1, 128]])
            nc.sync.dma_start(out=dst_ap, in_=eT_bf[:])

            # tW = text @ W1_bot (bf16), flattened the same way
            ptw = psH2.tile([N, D], F32, tag="h2p")
            for c in range(4):
                nc.tensor.matmul(
                    ptw[:], textT_bf[:, c, :], W1b_bf[:, c, :],
                    start=(c == 0), stop=(c == 3),
                )
            tW_bf = work.tile([N, D], BF16, tag="tW_bf")
            nc.scalar.copy(tW_bf[:], ptw[:])
            nc.sync.dma_start(out=tW_dram[br, :], in_=tW_bf[:])

            # rows 0..3 of the padded tiles (rows 4..127 stay zero)
            e4d = E4_dram[br, 0:1]
            nc.sync.dma_start(
                out=E4p[br][0:4, :],
                in_=bass.AP(tensor=e4d.tensor, offset=e4d.offset,
                            ap=[[512, 4], [2048, NG], [1, 512]]),
            )
            twd = tW_dram[br, 0:1]
            nc.sync.dma_start(
                out=tW4p[br][0:4, :],
                in_=bass.AP(tensor=twd.tensor, offset=twd.offset,
                            ap=[[512, 4], [2048, NG], [1, 512]]),
            )

        maccs = [
            const.tile([128, 2 * D], F32, tag=f"macc4_{i}", name=f"macc4_{i}")
            for i in range(2)
        ]

        def emit_layer3(br):
            # fold the 4 n-lanes inside the layer-3 matmul (K-accumulation),
            # so no vector fold sits on the critical path
            macc4 = maccs[br]
            po = psH1.tile([128, 128], F32, tag="h1p")
            for m in range(2):
                for q in range(4):
                    nc.tensor.matmul(
                        po[:], W3_s[:, m, :], macc4[:, ds(m * D + q * 128, 128)],
                        start=(m == 0 and q == 0), stop=(m == 1 and q == 3),
                        skip_group_check=True,
                    )
            outT = work.tile([128, 128], F32, tag="outT")
            if has_b3:
                nc.vector.tensor_scalar(
                    out=outT[:], in0=po[:], scalar1=1.0 / N, scalar2=b3t[:, 0:1],
                    op0=ALU.mult, op1=ALU.add,
                )
            else:
                nc.scalar.activation(outT[:], po[:], AF.Identity, scale=1.0 / N)
            nc.sync.dma_start(out=out_ext[br, :, :], in_=outT[:])

        # ---- per-branch prologue + main loop (branch 1's prologue is issued
        # after branch 0's loop so its scalar/PE work hides under the loop) --
        for br, text_s in enumerate((tn_s, ta_s)):
            branch_prologue(br, text_s)
            macc4 = maccs[br]

            # ---- main loop: 24 quads of n = {4g..4g+3} ---------------------
            for g in range(NG):
                if br == 1 and g == 3:
                    emit_layer3(0)  # branch-0 output, tucked into the loop
                h1a = hpool.tile([128, 2 * D], BF16, tag="h1a")
                h1b = hpool.tile([128, 2 * D], BF16, tag="h1b")
                for pair, h1x in ((0, h1a), (1, h1b)):
                    ph1 = psH1.tile([128, 2 * D], F32, tag="h1p")
                    for ci in range(2):
                        c = 2 * pair + ci
                        sl = ds(ci * D, D)
                        # broadcast aT (+b1) into PSUM bank via identity matmul
                        nc.tensor.matmul(
                            ph1[:, sl], I_bf[:], aT4_s[:, c, :],
                            start=True, stop=True,
                        )
                        # all 4 rank-1 updates in one K=4 matmul vs selector
                        nc.tensor.matmul(
                            ph1[:, sl], tW4p[br][:, ds(g * D + c * 128, 128)],
                            E4p[br][:, ts(g, 512)],
                            start=False, stop=True, skip_group_check=True,
                        )
                    # relu evictions split across Scalar (pair 0) / DVE (pair 1)
                    if pair == 0:
                        nc.scalar.activation(h1x[:], ph1[:], AF.Relu)
                    else:
                        nc.vector.tensor_scalar_max(h1x[:], ph1[:], 0.0)

                ph2 = psH2.tile([128, 2 * D], F32, tag="h2p")
                if has_b2:
                    for m in range(2):
                        nc.tensor.matmul(
                            ph2[:, ds(m * D, D)],
                            b2row_bf[0:1, ts(m, 128)], ones_bf[0:1, :],
                            start=True, stop=True,
                        )
                for c in range(4):
                    h1x = h1a if c < 2 else h1b
                    for m in range(2):
                        nc.tensor.matmul(
                            ph2[:, ds(m * D, D)],
                            W2_bf[:, c, ts(m, 128)],
                            h1x[:, ds((c % 2) * D, D)],
                            start=(c == 0 and not has_b2), stop=(c == 3),
                            skip_group_check=True,
                        )
                # fused: macc4 += relu(ph2)  (first group initializes)
                if g == 0:
                    nc.vector.tensor_scalar_max(macc4[:], ph2[:], 0.0)
                else:
                    nc.vector.scalar_tensor_tensor(
                        out=macc4[:], in0=ph2[:], scalar=0.0,
                        in1=macc4[:], op0=ALU.max, op1=ALU.add,
                    )

        emit_layer3(1)

    return nc


def make_in_maps(inputs):
    import ml_dtypes

    img = np.ascontiguousarray(np.asarray(inputs["img_embs"], np.float32))
    shared = {
        "tn": np.ascontiguousarray(np.asarray(inputs["normal_text_embs"], np.float32)),
        "ta": np.ascontiguousarray(np.asarray(inputs["abnormal_text_embs"], np.float32)),
        "W1": np.ascontiguousarray(np.asarray(inputs["W1"], np.float32)),
        "b1": np.ascontiguousarray(np.asarray(inputs["b1"], np.float32)),
        "W2": np.ascontiguousarray(np.asarray(inputs["W2"], np.float32)),
        "b2": np.ascontiguousarray(np.asarray(inputs["b2"], np.float32)),
        "W3": np.ascontiguousarray(np.asarray(inputs["W3"], np.float32)),
        "b3": np.ascontiguousarray(np.asarray(inputs["b3"], np.float32)),
        "idf": np.eye(128, dtype=np.float32),
        "idb": np.eye(128).astype(ml_dtypes.bfloat16),
        "zer": np.zeros(NG * 2048, dtype=ml_dtypes.bfloat16),
    }
    return [dict(shared, img=img[i * BL : (i + 1) * BL]) for i in range(NC)]


def kernel(**inputs) -> tuple:
    _install_bir_patch()

    scale = float(np.exp(np.asarray(inputs["logit_scale"], np.float32).reshape(-1)[0]))
    has_b1 = bool(np.any(np.asarray(inputs["b1"], np.float32)))
    has_b2 = bool(np.any(np.asarray(inputs["b2"], np.float32)))
    has_b3 = bool(np.any(np.asarray(inputs["b3"], np.float32)))

    nc = build_graph(scale, has_b1, has_b2, has_b3)
    in_maps = make_in_maps(inputs)
    res = run_bass_kernel_spmd(nc, in_maps, core_ids=list(range(NC)))
    h_n = np.concatenate([res.results[i]["out"][0].T for i in range(NC)], axis=0)
    h_a = np.concatenate([res.results[i]["out"][1].T for i in range(NC)], axis=0)
    return (h_n, h_a)


# revision 11
# speedup vs baseline: 1.0680x; 1.0680x over previous
"""AnoNAViLa forward kernel for 8 TRN2 NeuronCores (data-parallel over batch).

Math (per branch):
  sims = (img @ text.T) * scale;  w = softmax(sims);  e = exp(w)
  x = concat([img_rep, text * e[..., None]], -1)
  h = relu(x @ W1 + b1); h = relu(h @ W2 + b2); h = h @ W3 + b3
  out = h.mean(axis=1)

Key algebraic restructuring (exact, up to fp assoc):
  x @ W1 = img @ W1[:D] + e[b,n] * (text @ W1[D:])      (rank-1 per (b,n))
  mean_n (h2 @ W3 + b3) = (mean_n h2) @ W3 + b3          (mean before layer 3)

All weights/embeddings arrive pre-transposed/pre-cast from the host so the
device prologue is pure DMA + a short softmax chain.
"""
import sys

sys.path.insert(0, "/opt/trn_rl_repo")

from contextlib import ExitStack

import numpy as np
import orjson

import concourse.bass as bass
import concourse.mybir as mybir
import concourse.tile as tile
from concourse.bass import ds, ts
from concourse.bass_utils import run_bass_kernel_spmd

F32 = mybir.dt.float32
BF16 = mybir.dt.bfloat16
AF = mybir.ActivationFunctionType
ALU = mybir.AluOpType

NC = 8
B, N, D = 1024, 96, 512
BL = B // NC  # 128 rows per core
NG = N // 4  # 24 quad groups


# ---------------------------------------------------------------------------
# This walrus build rejects instructions with >1 semaphore wait/update
# ("Too many sync wait commands").  Split extras onto chained NoOps on the
# same engine (streams are in-order, so sequential waits == combined wait).
_bir_patch_installed = False


def _split_multi_sync(bir_json: bytes) -> bytes:
    d = orjson.loads(bir_json)
    ctr = [0]

    def mk_nop(inst, wait=None, update=None):
        ctr[0] += 1
        return {
            "debug": inst.get("debug", 0),
            "engine": inst["engine"],
            "ins": [],
            "outs": [],
            "name": f"{inst['name']}__ssplit{ctr[0]}",
            "opcode": "NoOp",
            "sync_info": {
                "on_update": [update] if update else [],
                "on_wait": [wait] if wait else [],
            },
        }

    changed = False
    for fn in d["functions"]:
        for bb in fn["blocks"]:
            new_insts = []
            for inst in bb["instructions"]:
                si = inst.get("sync_info")
                pre, post = [], []
                if si:
                    waits = si.get("on_wait") or []
                    if len(waits) > 1:
                        pre = [mk_nop(inst, wait=w) for w in waits[:-1]]
                        si["on_wait"] = [waits[-1]]
                        changed = True
                    upds = si.get("on_update") or []
                    if len(upds) > 1:
                        post = [mk_nop(inst, update=u) for u in upds[1:]]
                        si["on_update"] = [upds[0]]
                        changed = True
                new_insts.extend(pre)
                new_insts.append(inst)
                new_insts.extend(post)
            bb["instructions"] = new_insts
    return orjson.dumps(d) if changed else bir_json


def _install_bir_patch():
    global _bir_patch_installed
    if _bir_patch_installed:
        return
    _bir_patch_installed = True
    import concourse.bass_utils as bu
    import concourse.bass2jax as b2j

    orig = bu.compile_bir_kernel

    def patched(bir_json, tmpdir, neff_name="file.neff"):
        return orig(_split_multi_sync(bir_json), tmpdir, neff_name)

    bu.compile_bir_kernel = patched
    b2j.compile_bir_kernel = patched


# ---------------------------------------------------------------------------
def build_graph(scale: float, has_b1: bool, has_b2: bool, has_b3: bool) -> bass.Bass:
    nc = bass.Bass()

    # host-prepared layouts: [p, c, ...] with p the SBUF partition
    imgT_ext = nc.declare_dram_parameter("imgt", [128, 4, BL], F32, isOutput=False)
    tnT_ext = nc.declare_dram_parameter("tnt", [128, 4, N], F32, isOutput=False)
    taT_ext = nc.declare_dram_parameter("tat", [128, 4, N], F32, isOutput=False)
    W1_ext = nc.declare_dram_parameter("w1bf", [128, 8, D], BF16, isOutput=False)
    W2_ext = nc.declare_dram_parameter("w2bf", [128, 4, D // 2], BF16, isOutput=False)
    W3_ext = nc.declare_dram_parameter("w3r", [128, 2, D // 4], F32, isOutput=False)
    if has_b1:
        b1_ext = nc.declare_dram_parameter("b1t", [128, 4], F32, isOutput=False)
    if has_b2:
        b2_ext = nc.declare_dram_parameter("b2row", [1, D // 2], F32, isOutput=False)
    if has_b3:
        b3_ext = nc.declare_dram_parameter("b3t", [128, 1], F32, isOutput=False)
    idf_ext = nc.declare_dram_parameter("idf", [128, 128], F32, isOutput=False)
    idb_ext = nc.declare_dram_parameter("idb", [128, 128], BF16, isOutput=False)
    zer_ext = nc.declare_dram_parameter("zer", [4 * NG * 512], BF16, isOutput=False)
    out_ext = nc.declare_dram_parameter("out", [2, D // 4, BL], F32, isOutput=True)
    # DRAM bounce buffers for the cross-partition flatten of eT / tW.
    # E4_dram row k holds, per group g, e[b, 4g+k] at cols g*512 + k*128 + b
    # and zeros elsewhere (shared across branches; FIFO order on the sync
    # queue serializes scatter -> load -> next branch's scatter).
    E4_dram = nc.dram_tensor("E4_dram", [2, 4, NG * 512], BF16)
    tW_dram = nc.dram_tensor("tW_dram", [2, 4, NG * 512], BF16)

    with tile.TileContext(nc) as tc, ExitStack() as ctx:
        const = ctx.enter_context(tc.tile_pool(name="const", bufs=1))
        work = ctx.enter_context(tc.tile_pool(name="work", bufs=2))
        hpool = ctx.enter_context(tc.tile_pool(name="hpool", bufs=3))
        psH1 = ctx.enter_context(tc.tile_pool(name="psH1", bufs=2, space="PSUM"))
        psH2 = ctx.enter_context(tc.tile_pool(name="psH2", bufs=2, space="PSUM"))

        # ---- zero-padded K=128 selector tiles, one pair per branch ---------
        # rows 0..3 hold data; rows 4..127 zeroed once (f32 view: memset cost
        # is per-element, halving elements halves time).  Branch-0's pair is
        # split GpSimd/Vector so both finish ~13us; branch-1's pair follows.
        E4p = [const.tile([128, NG * 512], BF16, tag=f"E4p{i}", name=f"E4p{i}")
               for i in range(2)]
        tW4p = [const.tile([128, NG * 512], BF16, tag=f"tW4p{i}", name=f"tW4p{i}")
                for i in range(2)]
        nc.vector.memset(tW4p[0][:].bitcast(F32), 0.0)
        nc.gpsimd.memset(E4p[0][:].bitcast(F32), 0.0)
        nc.gpsimd.memset(E4p[1][:].bitcast(F32), 0.0)
        nc.gpsimd.memset(tW4p[1][:].bitcast(F32), 0.0)

        # ---- DMA loads (host already did all transposes/casts) -------------
        imgT_f = const.tile([128, 4, BL], F32)
        nc.sync.dma_start(out=imgT_f[:], in_=imgT_ext[:, :, :])
        for z in range(2):
            nc.sync.dma_start(out=E4_dram[z, :, :], in_=zer_ext[:])
        tnT_s = const.tile([128, 4, N], F32)
        nc.scalar.dma_start(out=tnT_s[:], in_=tnT_ext[:, :, :])
        taT_s = const.tile([128, 4, N], F32)
        nc.scalar.dma_start(out=taT_s[:], in_=taT_ext[:, :, :])
        I_s = const.tile([128, 128], F32)
        nc.gpsimd.dma_start(out=I_s[:], in_=idf_ext[:, :])
        I_bf = const.tile([128, 128], BF16)
        nc.scalar.dma_start(out=I_bf[:], in_=idb_ext[:, :])

        W1bf = const.tile([128, 8, D], BF16)
        nc.sync.dma_start(out=W1bf[:, 0:4, :], in_=W1_ext[:, 0:4, :])
        nc.scalar.dma_start(out=W1bf[:, 4:8, :], in_=W1_ext[:, 4:8, :])
        W2_bf = const.tile([128, 4, D // 2], BF16)
        nc.scalar.dma_start(out=W2_bf[:], in_=W2_ext[:, :, :])
        W3_s = const.tile([128, 2, D // 4], F32)
        nc.gpsimd.dma_start(out=W3_s[:], in_=W3_ext[:, :, :])
        if has_b1:
            b1t = const.tile([128, 4], F32)
            nc.gpsimd.dma_start(out=b1t[:], in_=b1_ext[:, :])
        if has_b2:
            b2row = const.tile([1, D // 2], F32)
            nc.gpsimd.dma_start(out=b2row[:], in_=b2_ext[:, :])
            b2row_bf = const.tile([1, D // 2], BF16)
            nc.gpsimd.tensor_copy(b2row_bf[:], b2row[:])
            ones_bf = const.tile([1, D], BF16)
            nc.gpsimd.memset(ones_bf[:], 1.0)
        if has_b3:
            b3t = const.tile([128, 1], F32)
            nc.gpsimd.dma_start(out=b3t[:], in_=b3_ext[:, :])

        imgT_bf = const.tile([128, 4, BL], BF16)
        nc.vector.tensor_copy(imgT_bf[:], imgT_f[:])

        # ---- aT4: img@W1_top + b1, transposed, replicated 4x along free ----
        aT4_s = const.tile([128, 4, 512], BF16)
        for co in range(4):
            pa = psH2.tile([128, 128], F32, tag="h2p")
            for ci in range(4):
                nc.tensor.matmul(
                    pa[:], W1bf[:, ci, ts(co, 128)], imgT_bf[:, ci, :],
                    start=(ci == 0), stop=(ci == 3),
                )
            if has_b1:
                nc.scalar.activation(
                    aT4_s[:, co, 0:128], pa[:], AF.Identity, bias=b1t[:, co : co + 1]
                )
            else:
                nc.scalar.copy(aT4_s[:, co, 0:128], pa[:])
            # replicate 4x along free via SBUF->SBUF broadcast DMA
            rep_src = aT4_s[:, co, 0:128]
            nc.gpsimd.dma_start(
                out=aT4_s[:, co, ds(128, 384)],
                in_=bass.AP(tensor=rep_src.tensor, offset=rep_src.offset,
                            ap=[rep_src.ap[0], [0, 3], [1, 128]]),
            )

        # ---- per-branch prologues (both upfront; PE work fills the memset
        # wait, selector loads land before the first quad) -------------------
        for br, textT_s in enumerate((tnT_s, taT_s)):
            textT_bf = work.tile([128, 4, N], BF16, tag="textT_bf")
            nc.vector.tensor_copy(textT_bf[:], textT_s[:])

            # sims = img @ text.T (scale folded into the exp activation)
            ps_sims = psH2.tile([BL, N], F32, tag="h2p")
            for c in range(4):
                nc.tensor.matmul(
                    ps_sims[:], imgT_f[:, c, :], textT_s[:, c, :],
                    start=(c == 0), stop=(c == 3),
                )
            # softmax over n then e = exp(w), all rowwise
            negmax = work.tile([BL, 1], F32, tag="negmax")
            nc.vector.tensor_reduce(
                negmax[:], ps_sims[:], axis=mybir.AxisListType.X, op=ALU.max,
                negate=True,
            )
            nb = work.tile([BL, 1], F32, tag="nb")
            nc.vector.tensor_scalar_mul(nb[:], negmax[:], float(scale))
            E_s = work.tile([BL, N], F32, tag="E_s")
            ssum = work.tile([BL, 1], F32, tag="ssum")
            nc.scalar.activation(
                E_s[:], ps_sims[:], AF.Exp, bias=nb[:, 0:1], scale=float(scale),
                accum_out=ssum[:, 0:1],
            )
            rr = work.tile([BL, 1], F32, tag="rr")
            nc.vector.reciprocal(rr[:], ssum[:])
            e_x = work.tile([BL, N], F32, tag="e_x")
            nc.scalar.activation(e_x[:], E_s[:], AF.Exp, scale=rr[:, 0:1])

            # eT (bf16), flatten rows onto partition 0 via DRAM bounce
            pe_t = psH1.tile([N, BL], F32, tag="h1p")
            nc.tensor.transpose(pe_t[:], e_x[:], I_s[:])
            eT_bf = work.tile([N, BL], BF16, tag="eT_bf")
            nc.scalar.copy(eT_bf[:], pe_t[:])
            # scatter e-rows onto the shared block-diagonal DRAM image, one
            # DMA per k so both sides stay single-partition-dim affine
            for k in range(4):
                src = eT_bf[:]
                nc.sync.dma_start(
                    out=bass.AP(tensor=E4_dram,
                                offset=(br * 4 + k) * (NG * 512) + k * 128,
                                ap=[[512, NG], [1, 128]]),
                    in_=bass.AP(tensor=src.tensor, offset=src.offset + k * BL,
                                ap=[[4 * BL, NG], [1, 128]]),
                )

            # tW = text @ W1_bot (bf16), flattened the same way
            ptw = psH2.tile([N, D], F32, tag="h2p")
            for c in range(4):
                nc.tensor.matmul(
                    ptw[:], textT_bf[:, c, :], W1bf[:, 4 + c, :],
                    start=(c == 0), stop=(c == 3),
                )
            tW_bf = work.tile([N, D], BF16, tag="tW_bf")
            nc.scalar.copy(tW_bf[:], ptw[:])
            for k in range(4):
                src = tW_bf[:]
                nc.sync.dma_start(
                    out=bass.AP(tensor=tW_dram,
                                offset=(br * 4 + k) * (NG * 512),
                                ap=[[512, NG], [1, 512]]),
                    in_=bass.AP(tensor=src.tensor, offset=src.offset + k * D,
                                ap=[[4 * D, NG], [1, 512]]),
                )

            # rows 0..3 of the padded tiles: 4 fat contiguous descriptors
            nc.sync.dma_start(
                out=E4p[br][0:4, :],
                in_=bass.AP(tensor=E4_dram, offset=br * 4 * NG * 512,
                            ap=[[NG * 512, 4], [1, NG * 512]]),
            )
            nc.sync.dma_start(
                out=tW4p[br][0:4, :],
                in_=bass.AP(tensor=tW_dram, offset=br * 4 * NG * 512,
                            ap=[[NG * 512, 4], [1, NG * 512]]),
            )

        maccs = [
            const.tile([128, 2 * D], F32, tag=f"macc4_{i}", name=f"macc4_{i}")
            for i in range(2)
        ]

        def emit_layer3(br):
            # fold the 4 n-lanes inside the layer-3 matmul (K-accumulation),
            # so no vector fold sits on the critical path
            macc4 = maccs[br]
            po = psH1.tile([128, 128], F32, tag="h1p")
            for m in range(2):
                for q in range(4):
                    nc.tensor.matmul(
                        po[:], W3_s[:, m, :], macc4[:, ds(m * D + q * 128, 128)],
                        start=(m == 0 and q == 0), stop=(m == 1 and q == 3),
                        skip_group_check=True,
                    )
            outT = work.tile([128, 128], F32, tag="outT")
            if has_b3:
                nc.vector.tensor_scalar(
                    out=outT[:], in0=po[:], scalar1=1.0 / N, scalar2=b3t[:, 0:1],
                    op0=ALU.mult, op1=ALU.add,
                )
            else:
                nc.scalar.activation(outT[:], po[:], AF.Identity, scale=1.0 / N)
            nc.sync.dma_start(out=out_ext[br, :, :], in_=outT[:])

        # ---- per-branch main loops -----------------------------------------
        for br in range(2):
            macc4 = maccs[br]
            for g in range(NG):
                if br == 1 and g == 3:
                    emit_layer3(0)  # branch-0 output, tucked into the loop
                h1a = hpool.tile([128, 2 * D], BF16, tag="h1a")
                h1b = hpool.tile([128, 2 * D], BF16, tag="h1b")
                for pair, h1x in ((0, h1a), (1, h1b)):
                    ph1 = psH1.tile([128, 2 * D], F32, tag="h1p")
                    for ci in range(2):
                        c = 2 * pair + ci
                        sl = ds(ci * D, D)
                        # broadcast aT (+b1) into PSUM bank via identity matmul
                        nc.tensor.matmul(
                            ph1[:, sl], I_bf[:], aT4_s[:, c, :],
                            start=True, stop=True,
                        )
                        # all 4 rank-1 updates in one K=4 matmul vs selector
                        nc.tensor.matmul(
                            ph1[:, sl], tW4p[br][:, ds(g * D + c * 128, 128)],
                            E4p[br][:, ts(g, 512)],
                            start=False, stop=True, skip_group_check=True,
                        )
                    # relu evictions split across Scalar (pair 0) / DVE (pair 1)
                    if pair == 0:
                        nc.scalar.activation(h1x[:], ph1[:], AF.Relu)
                    else:
                        nc.vector.tensor_scalar_max(h1x[:], ph1[:], 0.0)

                ph2 = psH2.tile([128, 2 * D], F32, tag="h2p")
                if has_b2:
                    for m in range(2):
                        nc.tensor.matmul(
                            ph2[:, ds(m * D, D)],
                            b2row_bf[0:1, ts(m, 128)], ones_bf[0:1, :],
                            start=True, stop=True,
                        )
                for c in range(4):
                    h1x = h1a if c < 2 else h1b
                    for m in range(2):
                        nc.tensor.matmul(
                            ph2[:, ds(m * D, D)],
                            W2_bf[:, c, ts(m, 128)],
                            h1x[:, ds((c % 2) * D, D)],
                            start=(c == 0 and not has_b2), stop=(c == 3),
                            skip_group_check=True,
                        )
                # fused: macc4 += relu(ph2)  (first group initializes)
                if g == 0:
                    nc.vector.tensor_scalar_max(macc4[:], ph2[:], 0.0)
                else:
                    nc.vector.scalar_tensor_tensor(
                        out=macc4[:], in0=ph2[:], scalar=0.0,
                        in1=macc4[:], op0=ALU.max, op1=ALU.add,
                    )

        emit_layer3(1)

    return nc


def make_in_maps(inputs):
    import ml_dtypes

    BF = ml_dtypes.bfloat16
    f32 = np.float32

    def rearr_w(w, p=128):
        # [C*p, d] -> [p, C, d]
        cpd = np.asarray(w, f32)
        c = cpd.shape[0] // p
        return np.ascontiguousarray(cpd.reshape(c, p, -1).transpose(1, 0, 2))

    def rearr_t(x):
        # [n, 4*128] -> [128, 4, n]  (transposed, chunked)
        xt = np.asarray(x, f32).T  # [512, n]
        return np.ascontiguousarray(xt.reshape(4, 128, -1).transpose(1, 0, 2))

    img = np.asarray(inputs["img_embs"], f32)
    b1 = np.asarray(inputs["b1"], f32)
    b2 = np.asarray(inputs["b2"], f32)
    b3 = np.asarray(inputs["b3"], f32)
    shared = {
        "tnt": rearr_t(inputs["normal_text_embs"]),
        "tat": rearr_t(inputs["abnormal_text_embs"]),
        "w1bf": rearr_w(inputs["W1"]).astype(BF),
        "w2bf": rearr_w(inputs["W2"]).astype(BF),
        "w3r": rearr_w(inputs["W3"]),
        "idf": np.eye(128, dtype=f32),
        "idb": np.eye(128).astype(BF),
        "zer": np.zeros(4 * NG * 512, dtype=BF),
    }
    if np.any(b1):
        shared["b1t"] = np.ascontiguousarray(b1.reshape(4, 128).T)
    if np.any(b2):
        shared["b2row"] = np.ascontiguousarray(b2.reshape(1, -1))
    if np.any(b3):
        shared["b3t"] = np.ascontiguousarray(b3.reshape(-1, 1))
    return [dict(shared, imgt=rearr_t(img[i * BL : (i + 1) * BL])) for i in range(NC)]


def kernel(**inputs) -> tuple:
    _install_bir_patch()

    scale = float(np.exp(np.asarray(inputs["logit_scale"], np.float32).reshape(-1)[0]))
    has_b1 = bool(np.any(np.asarray(inputs["b1"], np.float32)))
    has_b2 = bool(np.any(np.asarray(inputs["b2"], np.float32)))
    has_b3 = bool(np.any(np.asarray(inputs["b3"], np.float32)))

    nc = build_graph(scale, has_b1, has_b2, has_b3)
    in_maps = make_in_maps(inputs)
    res = run_bass_kernel_spmd(nc, in_maps, core_ids=list(range(NC)))
    h_n = np.concatenate([res.results[i]["out"][0].T for i in range(NC)], axis=0)
    h_a = np.concatenate([res.results[i]["out"][1].T for i in range(NC)], axis=0)
    return (h_n, h_a)


# revision 12
# speedup vs baseline: 1.0847x; 1.0156x over previous
"""AnoNAViLa forward kernel for 8 TRN2 NeuronCores (data-parallel over batch).

Math (per branch):
  sims = (img @ text.T) * scale;  w = softmax(sims);  e = exp(w)
  x = concat([img_rep, text * e[..., None]], -1)
  h = relu(x @ W1 + b1); h = relu(h @ W2 + b2); h = h @ W3 + b3
  out = h.mean(axis=1)

Key algebraic restructuring (exact, up to fp assoc):
  x @ W1 = img @ W1[:D] + e[b,n] * (text @ W1[D:])      (rank-1 per (b,n))
  mean_n (h2 @ W3 + b3) = (mean_n h2) @ W3 + b3          (mean before layer 3)

All weights/embeddings arrive pre-transposed/pre-cast from the host so the
device prologue is pure DMA + a short softmax chain.
"""
import sys

sys.path.insert(0, "/opt/trn_rl_repo")

from contextlib import ExitStack

import numpy as np
import orjson

import concourse.bass as bass
import concourse.mybir as mybir
import concourse.tile as tile
from concourse.bass import ds, ts
from concourse.bass_utils import run_bass_kernel_spmd

F32 = mybir.dt.float32
BF16 = mybir.dt.bfloat16
AF = mybir.ActivationFunctionType
ALU = mybir.AluOpType

NC = 8
B, N, D = 1024, 96, 512
BL = B // NC  # 128 rows per core
NG = N // 4  # 24 quad groups


# ---------------------------------------------------------------------------
# This walrus build rejects instructions with >1 semaphore wait/update
# ("Too many sync wait commands").  Split extras onto chained NoOps on the
# same engine (streams are in-order, so sequential waits == combined wait).
_bir_patch_installed = False


def _split_multi_sync(bir_json: bytes) -> bytes:
    d = orjson.loads(bir_json)
    ctr = [0]

    def mk_nop(inst, wait=None, update=None):
        ctr[0] += 1
        return {
            "debug": inst.get("debug", 0),
            "engine": inst["engine"],
            "ins": [],
            "outs": [],
            "name": f"{inst['name']}__ssplit{ctr[0]}",
            "opcode": "NoOp",
            "sync_info": {
                "on_update": [update] if update else [],
                "on_wait": [wait] if wait else [],
            },
        }

    changed = False
    for fn in d["functions"]:
        for bb in fn["blocks"]:
            new_insts = []
            for inst in bb["instructions"]:
                si = inst.get("sync_info")
                pre, post = [], []
                if si:
                    waits = si.get("on_wait") or []
                    if len(waits) > 1:
                        pre = [mk_nop(inst, wait=w) for w in waits[:-1]]
                        si["on_wait"] = [waits[-1]]
                        changed = True
                    upds = si.get("on_update") or []
                    if len(upds) > 1:
                        post = [mk_nop(inst, update=u) for u in upds[1:]]
                        si["on_update"] = [upds[0]]
                        changed = True
                new_insts.extend(pre)
                new_insts.append(inst)
                new_insts.extend(post)
            bb["instructions"] = new_insts
    return orjson.dumps(d) if changed else bir_json


def _install_bir_patch():
    global _bir_patch_installed
    if _bir_patch_installed:
        return
    _bir_patch_installed = True
    import concourse.bass_utils as bu
    import concourse.bass2jax as b2j

    orig = bu.compile_bir_kernel

    def patched(bir_json, tmpdir, neff_name="file.neff"):
        return orig(_split_multi_sync(bir_json), tmpdir, neff_name)

    bu.compile_bir_kernel = patched
    b2j.compile_bir_kernel = patched


# ---------------------------------------------------------------------------
def build_graph(scale: float, has_b1: bool, has_b2: bool, has_b3: bool) -> bass.Bass:
    nc = bass.Bass()

    # host-prepared layouts: [p, c, ...] with p the SBUF partition
    imgT_ext = nc.declare_dram_parameter("imgt", [128, 4, BL], F32, isOutput=False)
    tnT_ext = nc.declare_dram_parameter("tnt", [128, 4, N], F32, isOutput=False)
    taT_ext = nc.declare_dram_parameter("tat", [128, 4, N], F32, isOutput=False)
    W1_ext = nc.declare_dram_parameter("w1bf", [128, 8, D], BF16, isOutput=False)
    W2_ext = nc.declare_dram_parameter("w2bf", [128, 4, D // 2], BF16, isOutput=False)
    W3_ext = nc.declare_dram_parameter("w3r", [128, 2, D // 4], F32, isOutput=False)
    if has_b1:
        b1_ext = nc.declare_dram_parameter("b1t", [128, 4], F32, isOutput=False)
    if has_b2:
        b2_ext = nc.declare_dram_parameter("b2row", [1, D // 2], F32, isOutput=False)
    if has_b3:
        b3_ext = nc.declare_dram_parameter("b3t", [128, 1], F32, isOutput=False)
    idf_ext = nc.declare_dram_parameter("idf", [128, 128], F32, isOutput=False)
    idb_ext = nc.declare_dram_parameter("idb", [128, 128], BF16, isOutput=False)
    zer_ext = nc.declare_dram_parameter("zer", [4 * NG * 512], BF16, isOutput=False)
    out_ext = nc.declare_dram_parameter("out", [2, D // 4, BL], F32, isOutput=True)
    # DRAM bounce buffers for the cross-partition flatten of eT / tW.
    # E4_dram row k holds, per group g, e[b, 4g+k] at cols g*512 + k*128 + b
    # and zeros elsewhere (shared across branches; FIFO order on the sync
    # queue serializes scatter -> load -> next branch's scatter).
    E4_dram = nc.dram_tensor("E4_dram", [2, 4, NG * 512], BF16)
    tW_dram = nc.dram_tensor("tW_dram", [2, 4, NG * 512], BF16)

    with tile.TileContext(nc) as tc, ExitStack() as ctx:
        const = ctx.enter_context(tc.tile_pool(name="const", bufs=1))
        work = ctx.enter_context(tc.tile_pool(name="work", bufs=2))
        hpool = ctx.enter_context(tc.tile_pool(name="hpool", bufs=3))
        psH1 = ctx.enter_context(tc.tile_pool(name="psH1", bufs=2, space="PSUM"))
        psH2 = ctx.enter_context(tc.tile_pool(name="psH2", bufs=2, space="PSUM"))

        # ---- zero-padded K=128 selector tiles, one pair per branch ---------
        # rows 0..3 hold data; rows 4..127 zeroed once (f32 view: memset cost
        # is per-element, halving elements halves time).  Branch-0's pair is
        # split GpSimd/Vector so both finish ~13us; branch-1's pair follows.
        E4p = [const.tile([128, NG * 512], BF16, tag=f"E4p{i}", name=f"E4p{i}")
               for i in range(2)]
        tW4p = [const.tile([128, NG * 512], BF16, tag=f"tW4p{i}", name=f"tW4p{i}")
                for i in range(2)]
        nc.vector.memset(tW4p[0][:].bitcast(F32), 0.0)
        nc.gpsimd.memset(E4p[0][:].bitcast(F32), 0.0)
        nc.gpsimd.memset(E4p[1][:].bitcast(F32), 0.0)
        nc.gpsimd.memset(tW4p[1][:].bitcast(F32), 0.0)

        # ---- DMA loads (host already did all transposes/casts) -------------
        imgT_f = const.tile([128, 4, BL], F32)
        nc.sync.dma_start(out=imgT_f[:], in_=imgT_ext[:, :, :])
        for z in range(2):
            nc.gpsimd.dma_start(out=E4_dram[z, :, :], in_=zer_ext[:])
        tnT_s = const.tile([128, 4, N], F32)
        nc.scalar.dma_start(out=tnT_s[:], in_=tnT_ext[:, :, :])
        taT_s = const.tile([128, 4, N], F32)
        nc.scalar.dma_start(out=taT_s[:], in_=taT_ext[:, :, :])
        I_s = const.tile([128, 128], F32)
        nc.gpsimd.dma_start(out=I_s[:], in_=idf_ext[:, :])
        I_bf = const.tile([128, 128], BF16)
        nc.gpsimd.dma_start(out=I_bf[:], in_=idb_ext[:, :])

        W1bf = const.tile([128, 8, D], BF16)
        nc.scalar.dma_start(out=W1bf[:, 0:4, :], in_=W1_ext[:, 0:4, :])
        nc.scalar.dma_start(out=W1bf[:, 4:6, :], in_=W1_ext[:, 4:6, :])
        nc.sync.dma_start(out=W1bf[:, 6:8, :], in_=W1_ext[:, 6:8, :])
        W2_bf = const.tile([128, 4, D // 2], BF16)
        nc.scalar.dma_start(out=W2_bf[:], in_=W2_ext[:, :, :])
        W3_s = const.tile([128, 2, D // 4], F32)
        nc.gpsimd.dma_start(out=W3_s[:], in_=W3_ext[:, :, :])
        if has_b1:
            b1t = const.tile([128, 4], F32)
            nc.gpsimd.dma_start(out=b1t[:], in_=b1_ext[:, :])
        if has_b2:
            b2row = const.tile([1, D // 2], F32)
            nc.gpsimd.dma_start(out=b2row[:], in_=b2_ext[:, :])
            b2row_bf = const.tile([1, D // 2], BF16)
            nc.gpsimd.tensor_copy(b2row_bf[:], b2row[:])
            ones_bf = const.tile([1, D], BF16)
            nc.gpsimd.memset(ones_bf[:], 1.0)
        if has_b3:
            b3t = const.tile([128, 1], F32)
            nc.gpsimd.dma_start(out=b3t[:], in_=b3_ext[:, :])

        imgT_bf = const.tile([128, 4, BL], BF16)
        nc.scalar.copy(imgT_bf[:], imgT_f[:])

        # ---- aT4: img@W1_top + b1, transposed, replicated 4x along free ----
        aT4_s = const.tile([128, 4, 512], BF16)
        for co in range(4):
            pa = psH2.tile([128, 128], F32, tag="h2p")
            for ci in range(4):
                nc.tensor.matmul(
                    pa[:], W1bf[:, ci, ts(co, 128)], imgT_bf[:, ci, :],
                    start=(ci == 0), stop=(ci == 3),
                )
            if has_b1:
                nc.scalar.activation(
                    aT4_s[:, co, 0:128], pa[:], AF.Identity, bias=b1t[:, co : co + 1]
                )
            else:
                nc.scalar.copy(aT4_s[:, co, 0:128], pa[:])
            # replicate 4x along free via SBUF->SBUF broadcast DMA
            rep_src = aT4_s[:, co, 0:128]
            nc.gpsimd.dma_start(
                out=aT4_s[:, co, ds(128, 384)],
                in_=bass.AP(tensor=rep_src.tensor, offset=rep_src.offset,
                            ap=[rep_src.ap[0], [0, 3], [1, 128]]),
            )

        # ---- per-branch prologues (both upfront; PE work fills the memset
        # wait, selector loads land before the first quad) -------------------
        for br, textT_s in enumerate((tnT_s, taT_s)):
            textT_bf = work.tile([128, 4, N], BF16, tag="textT_bf")
            nc.scalar.copy(textT_bf[:], textT_s[:])

            # sims = img @ text.T (scale folded into the exp activation)
            ps_sims = psH2.tile([BL, N], F32, tag="h2p")
            for c in range(4):
                nc.tensor.matmul(
                    ps_sims[:], imgT_f[:, c, :], textT_s[:, c, :],
                    start=(c == 0), stop=(c == 3),
                )
            # softmax over n then e = exp(w), all rowwise
            negmax = work.tile([BL, 1], F32, tag="negmax")
            nc.vector.tensor_reduce(
                negmax[:], ps_sims[:], axis=mybir.AxisListType.X, op=ALU.max,
                negate=True,
            )
            nb = work.tile([BL, 1], F32, tag="nb")
            nc.vector.tensor_scalar_mul(nb[:], negmax[:], float(scale))
            E_s = work.tile([BL, N], F32, tag="E_s")
            ssum = work.tile([BL, 1], F32, tag="ssum")
            nc.scalar.activation(
                E_s[:], ps_sims[:], AF.Exp, bias=nb[:, 0:1], scale=float(scale),
                accum_out=ssum[:, 0:1],
            )
            rr = work.tile([BL, 1], F32, tag="rr")
            nc.vector.reciprocal(rr[:], ssum[:])
            e_x = work.tile([BL, N], F32, tag="e_x")
            nc.scalar.activation(e_x[:], E_s[:], AF.Exp, scale=rr[:, 0:1])

            # eT (bf16), flatten rows onto partition 0 via DRAM bounce
            pe_t = psH1.tile([N, BL], F32, tag="h1p")
            nc.tensor.transpose(pe_t[:], e_x[:], I_s[:])
            eT_bf = work.tile([N, BL], BF16, tag="eT_bf")
            nc.scalar.copy(eT_bf[:], pe_t[:])
            # scatter e-rows onto the shared block-diagonal DRAM image, one
            # DMA per k so both sides stay single-partition-dim affine
            for k in range(4):
                src = eT_bf[:]
                nc.sync.dma_start(
                    out=bass.AP(tensor=E4_dram,
                                offset=(br * 4 + k) * (NG * 512) + k * 128,
                                ap=[[512, NG], [1, 128]]),
                    in_=bass.AP(tensor=src.tensor, offset=src.offset + k * BL,
                                ap=[[4 * BL, NG], [1, 128]]),
                )

            # tW = text @ W1_bot (bf16), flattened the same way
            ptw = psH2.tile([N, D], F32, tag="h2p")
            for c in range(4):
                nc.tensor.matmul(
                    ptw[:], textT_bf[:, c, :], W1bf[:, 4 + c, :],
                    start=(c == 0), stop=(c == 3),
                )
            tW_bf = work.tile([N, D], BF16, tag="tW_bf")
            nc.scalar.copy(tW_bf[:], ptw[:])
            for k in range(4):
                src = tW_bf[:]
                nc.sync.dma_start(
                    out=bass.AP(tensor=tW_dram,
                                offset=(br * 4 + k) * (NG * 512),
                                ap=[[512, NG], [1, 512]]),
                    in_=bass.AP(tensor=src.tensor, offset=src.offset + k * D,
                                ap=[[4 * D, NG], [1, 512]]),
                )

            # rows 0..3 of the padded tiles: 4 fat contiguous descriptors
            nc.sync.dma_start(
                out=E4p[br][0:4, :],
                in_=bass.AP(tensor=E4_dram, offset=br * 4 * NG * 512,
                            ap=[[NG * 512, 4], [1, NG * 512]]),
            )
            nc.sync.dma_start(
                out=tW4p[br][0:4, :],
                in_=bass.AP(tensor=tW_dram, offset=br * 4 * NG * 512,
                            ap=[[NG * 512, 4], [1, NG * 512]]),
            )

        maccs = [
            const.tile([128, 2 * D], F32, tag=f"macc4_{i}", name=f"macc4_{i}")
            for i in range(2)
        ]

        def emit_layer3(br):
            # fold the 4 n-lanes inside the layer-3 matmul (K-accumulation),
            # so no vector fold sits on the critical path
            macc4 = maccs[br]
            po = psH1.tile([128, 128], F32, tag="h1p")
            for m in range(2):
                for q in range(4):
                    nc.tensor.matmul(
                        po[:], W3_s[:, m, :], macc4[:, ds(m * D + q * 128, 128)],
                        start=(m == 0 and q == 0), stop=(m == 1 and q == 3),
                        skip_group_check=True,
                    )
            outT = work.tile([128, 128], F32, tag="outT")
            if has_b3:
                nc.vector.tensor_scalar(
                    out=outT[:], in0=po[:], scalar1=1.0 / N, scalar2=b3t[:, 0:1],
                    op0=ALU.mult, op1=ALU.add,
                )
            else:
                nc.scalar.activation(outT[:], po[:], AF.Identity, scale=1.0 / N)
            nc.sync.dma_start(out=out_ext[br, :, :], in_=outT[:])

        # ---- per-branch main loops -----------------------------------------
        for br in range(2):
            macc4 = maccs[br]
            for g in range(NG):
                if br == 1 and g == 3:
                    emit_layer3(0)  # branch-0 output, tucked into the loop
                h1a = hpool.tile([128, 2 * D], BF16, tag="h1a")
                h1b = hpool.tile([128, 2 * D], BF16, tag="h1b")
                for pair, h1x in ((0, h1a), (1, h1b)):
                    ph1 = psH1.tile([128, 2 * D], F32, tag="h1p")
                    for ci in range(2):
                        c = 2 * pair + ci
                        sl = ds(ci * D, D)
                        # broadcast aT (+b1) into PSUM bank via identity matmul
                        nc.tensor.matmul(
                            ph1[:, sl], I_bf[:], aT4_s[:, c, :],
                            start=True, stop=True,
                        )
                        # all 4 rank-1 updates in one K=4 matmul vs selector
                        nc.tensor.matmul(
                            ph1[:, sl], tW4p[br][:, ds(g * D + c * 128, 128)],
                            E4p[br][:, ts(g, 512)],
                            start=False, stop=True, skip_group_check=True,
                        )
                    # relu evictions split across Scalar (pair 0) / DVE (pair 1)
                    if pair == 0:
                        nc.scalar.activation(h1x[:], ph1[:], AF.Relu)
                    else:
                        nc.vector.tensor_scalar_max(h1x[:], ph1[:], 0.0)

                ph2 = psH2.tile([128, 2 * D], F32, tag="h2p")
                if has_b2:
                    for m in range(2):
                        nc.tensor.matmul(
                            ph2[:, ds(m * D, D)],
                            b2row_bf[0:1, ts(m, 128)], ones_bf[0:1, :],
                            start=True, stop=True,
                        )
                for c in range(4):
                    h1x = h1a if c < 2 else h1b
                    for m in range(2):
                        nc.tensor.matmul(
                            ph2[:, ds(m * D, D)],
                            W2_bf[:, c, ts(m, 128)],
                            h1x[:, ds((c % 2) * D, D)],
                            start=(c == 0 and not has_b2), stop=(c == 3),
                            skip_group_check=True,
                        )
                # fused: macc4 += relu(ph2)  (first group initializes)
                if g == 0:
                    nc.vector.tensor_scalar_max(macc4[:], ph2[:], 0.0)
                else:
                    nc.vector.scalar_tensor_tensor(
                        out=macc4[:], in0=ph2[:], scalar=0.0,
                        in1=macc4[:], op0=ALU.max, op1=ALU.add,
                    )

        emit_layer3(1)

    return nc


def make_in_maps(inputs):
    import ml_dtypes

    BF = ml_dtypes.bfloat16
    f32 = np.float32

    def rearr_w(w, p=128):
        # [C*p, d] -> [p, C, d]
        cpd = np.asarray(w, f32)
        c = cpd.shape[0] // p
        return np.ascontiguousarray(cpd.reshape(c, p, -1).transpose(1, 0, 2))

    def rearr_t(x):
        # [n, 4*128] -> [128, 4, n]  (transposed, chunked)
        xt = np.asarray(x, f32).T  # [512, n]
        return np.ascontiguousarray(xt.reshape(4, 128, -1).transpose(1, 0, 2))

    img = np.asarray(inputs["img_embs"], f32)
    b1 = np.asarray(inputs["b1"], f32)
    b2 = np.asarray(inputs["b2"], f32)
    b3 = np.asarray(inputs["b3"], f32)
    shared = {
        "tnt": rearr_t(inputs["normal_text_embs"]),
        "tat": rearr_t(inputs["abnormal_text_embs"]),
        "w1bf": rearr_w(inputs["W1"]).astype(BF),
        "w2bf": rearr_w(inputs["W2"]).astype(BF),
        "w3r": rearr_w(inputs["W3"]),
        "idf": np.eye(128, dtype=f32),
        "idb": np.eye(128).astype(BF),
        "zer": np.zeros(4 * NG * 512, dtype=BF),
    }
    if np.any(b1):
        shared["b1t"] = np.ascontiguousarray(b1.reshape(4, 128).T)
    if np.any(b2):
        shared["b2row"] = np.ascontiguousarray(b2.reshape(1, -1))
    if np.any(b3):
        shared["b3t"] = np.ascontiguousarray(b3.reshape(-1, 1))
    return [dict(shared, imgt=rearr_t(img[i * BL : (i + 1) * BL])) for i in range(NC)]


def kernel(**inputs) -> tuple:
    _install_bir_patch()

    scale = float(np.exp(np.asarray(inputs["logit_scale"], np.float32).reshape(-1)[0]))
    has_b1 = bool(np.any(np.asarray(inputs["b1"], np.float32)))
    has_b2 = bool(np.any(np.asarray(inputs["b2"], np.float32)))
    has_b3 = bool(np.any(np.asarray(inputs["b3"], np.float32)))

    nc = build_graph(scale, has_b1, has_b2, has_b3)
    in_maps = make_in_maps(inputs)
    res = run_bass_kernel_spmd(nc, in_maps, core_ids=list(range(NC)))
    h_n = np.concatenate([res.results[i]["out"][0].T for i in range(NC)], axis=0)
    h_a = np.concatenate([res.results[i]["out"][1].T for i in range(NC)], axis=0)
    return (h_n, h_a)
